# revision 76
# baseline (speedup 1.0000x reference)
"""Trainium2 Bass kernel for sparse (top-k) multi-head causal attention.

Problem (hardcoded shapes, from the reference):
  B=32, S=512, D=512, H=8, DK=64, k_index=5 (any k<=8 supported)
  out = TopKCausalAttention(q, k, v; w_q..w_o, b_q..b_o)

Sharding: data-parallel over batch across 8 NeuronCores (4 batches/core).

v2 design notes (cost-model-driven; the graded metric is TimelineSim):
  - Matmul cost = output-free-dim rows x cycles/row(dtype). fp32 = 4.0,
    float32r = 1.0 when N >= 256 (HW-measured: f32r = fp32 rounded to 11
    explicit mantissa bits), fp16/bf16 = 1.0.
  - score path (q/k proj + QK^T) runs in f32r (score_mode="f32r") or in a
    3-term fp16 hi/lo split (score_mode="f16x2", ~fp26 effective).
  - Per (head, row-tile): QK^T+causal-mask -> PSUM, ACT exp -> e (f32 SBUF,
    PSUM freed immediately), DVE max8 -> top8.  Z = reduce_sum(top8[:k])
    (for rows < k the top-k slots hold every valid entry plus zeros, so
    this equals the full row sum the reference uses); rows < k keep
    everything via tau := 0; row 0's Z memset to 1 (its p row is all
    zeros).  es = e * (1/Z) on GPSIMD (SBUF-only engine), then one DVE
    scalar_tensor_tensor p = (e >= tau) * es gives normalized fp16 probs
    with an exact fp32 selection compare.
  - The per-(hp, ri) chains are software-pipelined (A1 = QK/exp/max8,
    A2 = Z/select, B = transpose/attnT) with v-projection groups as PE
    filler; both heads' pT blocks pack into one 2-bank fp16 PSUM tile
    with a single wide evacuation per column tile.  attnT and the output
    projection run in fp16.
  - Engine constraints found the hard way: GPSIMD has no PSUM access and
    rejects scalar_tensor_tensor at codegen; DMA cannot read PSUM;
    PE transpose-mode IGNORES the identity operand's values.
"""

import math
import os

os.environ.setdefault("MYCRO_LOCAL_CACHE", "1")

from contextlib import ExitStack

import numpy as np

import concourse.bass as bass
import concourse.bacc as bacc
import concourse.mybir as mybir
import concourse.tile as tile
from concourse.bass_utils import run_bass_kernel_spmd

B, S, D, H = 32, 512, 512, 8
DK = D // H  # 64
NCORES = 8
BC = B // NCORES  # batches per core
RT = S // 128  # row tiles per sequence
FT = D // 128  # feature tiles
NEG = -1.0e32

F32 = mybir.dt.float32
F32R = mybir.dt.float32r
BF16 = mybir.dt.bfloat16
F16 = mybir.dt.float16

_last_nc = None

CFG = {
    "score_mode": os.environ.get("SCORE_MODE", "f32r"),  # f32r | f16x2
    "trace": False,
}


def _build_program(k_index: int, has_bias: dict, mode: str):
    nc = bacc.Bacc(
        "TRN2", target_bir_lowering=False, debug=False, num_devices=NCORES
    )

    split = mode == "f16x2"
    QDT = F16 if split else F32R

    # --- DRAM I/O -------------------------------------------------------
    if split:
        qTh = nc.dram_tensor("qTh", (BC, D, S), F16, kind="ExternalInput").ap()
        qTl = nc.dram_tensor("qTl", (BC, D, S), F16, kind="ExternalInput").ap()
        kTh = nc.dram_tensor("kTh", (BC, D, S), F16, kind="ExternalInput").ap()
        kTl = nc.dram_tensor("kTl", (BC, D, S), F16, kind="ExternalInput").ap()
        wqh = nc.dram_tensor("wqh", (D, D), F16, kind="ExternalInput").ap()
        wql = nc.dram_tensor("wql", (D, D), F16, kind="ExternalInput").ap()
        wkh = nc.dram_tensor("wkh", (D, D), F16, kind="ExternalInput").ap()
        wkl = nc.dram_tensor("wkl", (D, D), F16, kind="ExternalInput").ap()
    else:
        qT = nc.dram_tensor("qT", (BC, D, S), QDT, kind="ExternalInput").ap()
        kT = nc.dram_tensor("kT", (BC, D, S), QDT, kind="ExternalInput").ap()
        wq = nc.dram_tensor("wq", (D, D), QDT, kind="ExternalInput").ap()
        wk = nc.dram_tensor("wk", (D, D), QDT, kind="ExternalInput").ap()
    vT = nc.dram_tensor("vT", (BC, D, S), F16, kind="ExternalInput").ap()
    wv = nc.dram_tensor("wv", (D, D), F16, kind="ExternalInput").ap()
    wo = nc.dram_tensor("wo", (D, D), F16, kind="ExternalInput").ap()
    bias_aps = {}
    for name in ("bq", "bk", "bv", "bo"):
        if has_bias[name]:
            bias_aps[name] = nc.dram_tensor(
                name, (1, D), F32, kind="ExternalInput"
            ).ap()
    out = nc.dram_tensor("out", (BC, S, D), F32, kind="ExternalOutput").ap()

    # --- inline constants ----------------------------------------------
    ident_np = np.eye(128, dtype=np.float32)
    mask_np = np.where(
        np.arange(128)[None, :] >= np.arange(128)[:, None], NEG, 0.0
    ).astype(np.float32)
    ident_p = nc.inline_tensor(
        ident_np.astype(mybir.dt.np(F16)), name="identp"
    ).ap()
    ident_b = nc.inline_tensor(
        ident_np.astype(mybir.dt.np(BF16)), name="identb"
    ).ap()
    maskT_b = nc.inline_tensor(
        mask_np.T.copy().astype(mybir.dt.np(BF16)), name="maskT"
    ).ap()
    ones_row = nc.inline_tensor(
        np.ones((1, S), dtype=np.float32), name="onesrow"
    ).ap()

    with tile.TileContext(nc) as tc, ExitStack() as ctx:
        # ---------------- pools ----------------
        consts = ctx.enter_context(tc.tile_pool(name="consts", bufs=1))
        xpool = ctx.enter_context(tc.tile_pool(name="xpool", bufs=2))
        projpool = ctx.enter_context(tc.tile_pool(name="projpool", bufs=2))
        epool = ctx.enter_context(tc.tile_pool(name="epool", bufs=16))
        pupool = ctx.enter_context(tc.tile_pool(name="pupool", bufs=18))
        ptpool = ctx.enter_context(tc.tile_pool(name="ptpool", bufs=8))
        espool = ctx.enter_context(tc.tile_pool(name="espool", bufs=10))
        smallpool = ctx.enter_context(tc.tile_pool(name="smallpool", bufs=4))
        atpool = ctx.enter_context(tc.tile_pool(name="atpool", bufs=3))
        ypool = ctx.enter_context(tc.tile_pool(name="ypool", bufs=3))

        ps_proj = ctx.enter_context(
            tc.tile_pool(name="ps_proj", bufs=2, space="PSUM"))
        ps_sc = ctx.enter_context(
            tc.tile_pool(name="ps_sc", bufs=3, space="PSUM"))
        ps_pt = ctx.enter_context(
            tc.tile_pool(name="ps_pt", bufs=2, space="PSUM"))
        ps_aty = ctx.enter_context(
            tc.tile_pool(name="ps_aty", bufs=1, space="PSUM"))

        # ---------------- resident constants ----------------
        # q/k weights first, then batch 0's activations, then the rest:
        # first projection matmuls start as early as possible.
        def load_w(ap, name):
            t = consts.tile_from(
                ap.rearrange("(f p) s -> p f s", p=128), name=name)
            return [t[:, f, :] for f in range(FT)]

        def load_w_half(ap, name, half):
            """dt-half of a weight: columns [half*256, half*256+256)."""
            t = consts.tile_from(
                ap.rearrange("(f p) s -> p f s", p=128)[
                    :, :, half * 256:(half + 1) * 256],
                name=f"{name}h{half}")
            return t

        class WHalves:
            """w_sb[f][:, dt*128:(dt+1)*128]-compatible accessor over two
            half-tiles loaded separately (first half arrives sooner)."""
            def __init__(self, h0, h1):
                self.h = (h0, h1)

            def slice(self, f, dt):
                return self.h[dt // 2][:, f, (dt % 2) * 128:(dt % 2 + 1) * 128]

        if split:
            wq_sb = list(zip(load_w(wqh, "wqh"), load_w(wql, "wql")))
        else:
            wq_h0 = load_w_half(wq, "wq", 0)

        def load_merged(ap_b, name):
            """One DMA for a [D, S] DRAM slab -> SBUF [128, FT*S]; chunk f
            of the input-feature dim lives at free offset f*S."""
            return [xpool.tile_from(ap_b[f * 128:(f + 1) * 128, :],
                                    name=f"{name}{f}") for f in range(FT)]

        def load_x(b):
            if split:
                xq = list(zip(load_merged(qTh[b], "xqh"),
                              load_merged(qTl[b], "xql")))
                xk = list(zip(load_merged(kTh[b], "xkh"),
                              load_merged(kTl[b], "xkl")))
            else:
                xq = load_merged(qT[b], "xq")
                xk = load_merged(kT[b], "xk")
            xv = load_merged(vT[b], "xv")
            return xq, xk, xv

        _xq0 = None
        if split:
            wk_sb = list(zip(load_w(wkh, "wkh"), load_w(wkl, "wkl")))
            preloaded = {0: load_x(0)}
        else:
            xq0 = load_merged(qT[0], "xq")
            wk_h0 = load_w_half(wk, "wk", 0)
            xk0 = load_merged(kT[0], "xk")
            wq_sb = WHalves(wq_h0, load_w_half(wq, "wq", 1))
            wk_sb = WHalves(wk_h0, load_w_half(wk, "wk", 1))
            preloaded = {0: (xq0, xk0, load_merged(vT[0], "xv"))}
        identb_sb = consts.tile_from(ident_b, name="identb_sb")
        maskT_sb = consts.tile_from(maskT_b, name="maskT_sb")
        wv_sb = load_w(wv, "wv")
        identp_sb = consts.tile_from(ident_p, name="identp_sb")
        wo_sb = load_w(wo, "wo")
        ones_sb = consts.tile_from(ones_row, name="ones_sb")
        bias_sb = {
            nm: consts.tile_from(ap, name=f"{nm}_sb")
            for nm, ap in bias_aps.items()
        }

        Exp = mybir.ActivationFunctionType.Exp
        AO = mybir.AluOpType

        def emit_qk_psum(ps, w_t, x_t, dt, bkey):
            """Accumulate one projection output tile [128(dt), S] in PSUM."""
            nbias = bkey in bias_sb
            if split:
                n = 3 * FT
                i = 0
                for f in range(FT):
                    (wh, wl), (xh, xl) = w_t[f], x_t[f]
                    for lw, lx in ((wh, xh), (wh, xl), (wl, xh)):
                        nc.tensor.matmul(
                            ps, lw[:, dt * 128:(dt + 1) * 128], lx,
                            start=(i == 0), stop=(i == n - 1 and not nbias))
                        i += 1
            else:
                for f in range(FT):
                    nc.tensor.matmul(
                        ps, w_t.slice(f, dt), x_t[f],
                        start=(f == 0), stop=(f == FT - 1 and not nbias))
            if nbias:
                nc.tensor.matmul(
                    ps, bias_sb[bkey][0:1, dt * 128:(dt + 1) * 128],
                    ones_sb, start=False, stop=True)

        def emit_proj(b):
            """Loads + q/k/v projections for batch b."""
            if b in preloaded:
                xq, xk, xv = preloaded.pop(b)
            else:
                xq, xk, xv = load_x(b)
            qhT, khT, vh = [], [], []
            for dt in range(FT):
                for which, w_t, x_t, bkey, outl in (
                        ("q", wq_sb, xq, "bq", qhT),
                        ("k", wk_sb, xk, "bk", khT)):
                    ps = ps_proj.tile([128, S], F32, name="psp", tag="psproj")
                    emit_qk_psum(ps, w_t, x_t, dt, bkey)
                    if split:
                        hi = projpool.tile([128, S], F16, name=f"{which}h{dt}",
                                           tag=f"{which}h{dt}")
                        nc.scalar.copy(hi, ps)
                        lo = projpool.tile([128, S], F16, name=f"{which}l{dt}",
                                           tag=f"{which}l{dt}")
                        nc.vector.tensor_tensor(lo, ps, hi, op=AO.subtract)
                        outl.append((hi, lo))
                    else:
                        t = projpool.tile([128, S], QDT, name=f"{which}hT{dt}",
                                          tag=f"{which}hT{dt}")
                        nc.scalar.copy(t, ps)
                        outl.append(t)
            def do_v(rt):
                ps = ps_proj.tile([128, D], F32, name="psv", tag="psproj")
                nbias = "bv" in bias_sb
                for f in range(FT):
                    nc.tensor.matmul(
                        ps, xv[f][:, rt * 128:(rt + 1) * 128], wv_sb[f],
                        start=(f == 0), stop=(f == FT - 1 and not nbias))
                if nbias:
                    nc.tensor.matmul(
                        ps, ones_sb[0:1, 0:128], bias_sb["bv"],
                        start=False, stop=True)
                t = projpool.tile([128, D], F16, name=f"vh{rt}", tag=f"vh{rt}")
                nc.scalar.copy(t, ps)
                vh.append(t)
            return qhT, khT, vh, do_v

        def stage_a1(hp, qhT, khT):
            """QK / exp / top8 / select / normalize chain for head pair hp.

            Heads 2*hp, 2*hp+1 live on partition halves 0:64 / 64:128 of
            qhT[hp] / khT[hp].  Returns the normalized p tiles.
            """
            top8 = smallpool.tile([128, 2 * RT * 8], F32, name="top8",
                                  tag="top8")
            zks = smallpool.tile([128, 2 * RT], F32, name="zks", tag="zks")
            es_ = [[None] * RT, [None] * RT]
            st = (top8, zks, es_)
            for ri in range(RT):
                w = (ri + 1) * 128
                wmm = max(w, 256) if not split else w
                spss = []
                for hh in range(2):
                    po = hh * 64
                    sps = ps_sc.tile([128, S], F32, name="sps", tag="sps")
                    if split:
                        (qh, ql) = qhT[hp]
                        (kh, kl) = khT[hp]
                        terms = ((qh, kh), (qh, kl), (ql, kh))
                        for i, (lq, lk) in enumerate(terms):
                            nc.tensor.matmul(
                                sps[:, 0:w],
                                lq[po:po + 64, ri * 128:(ri + 1) * 128],
                                lk[po:po + 64, 0:w],
                                start=(i == 0), stop=False)
                    else:
                        nc.tensor.matmul(
                            sps[:, 0:wmm],
                            qhT[hp][po:po + 64, ri * 128:(ri + 1) * 128],
                            khT[hp][po:po + 64, 0:wmm],
                            start=True, stop=False)
                    spss.append(sps)
                for hh in range(2):
                    # strictly-causal additive mask on the diagonal tile
                    nc.tensor.matmul(
                        spss[hh][:, ri * 128:(ri + 1) * 128],
                        maskT_sb, identb_sb, start=False, stop=True)
                for hh in range(2):
                    w8 = ri * 16 + hh * 8
                    e = epool.tile([128, S], F32, name="e", tag="e")
                    nc.scalar.activation(e[:, 0:w], spss[hh][:, 0:w], Exp)
                    nc.vector.max(out=top8[:, w8:w8 + 8], in_=e[:, 0:w])
                    if ri == 0:
                        # rows < k keep every valid entry: tau := 0
                        nc.vector.memset(
                            top8[0:k_index, w8 + k_index - 1:w8 + k_index],
                            0.0)
                    # keep-mask on Pool: exact f32 compare, fp16 0/1 out;
                    # no rz dependency so Pool's work spreads into A1
                    m = espool.tile([128, S], F16, name="m", tag="es")
                    nc.gpsimd.tensor_scalar(
                        m[:, 0:w], e[:, 0:w],
                        top8[:, w8 + k_index - 1:w8 + k_index],
                        None, op0=AO.is_ge)
                    es_[hh][ri] = (e, m)
            return st

        def stage_a2(hp, st):
            top8, zks, es_ = st
            # Z = sum of the top-k kept values, straight from top8 (for
            # rows < k the top-k slots hold every valid entry plus zeros,
            # so this is the full row sum as the reference requires)
            nc.vector.reduce_sum(
                zks, top8.rearrange("p (g e) -> p g e", e=8)[:, :, 0:k_index],
                axis=mybir.AxisListType.X)
            # row 0 is fully masked: its p row is all zeros; avoid 1/0
            nc.vector.memset(zks[0:1, 0:2], 1.0)
            rz = smallpool.tile([128, 2 * RT], F32, name="rz", tag="rz")
            nc.vector.reciprocal(rz, zks)
            # pn = (e * rz) * m in one DVE pass; m carried the exact
            # selection, so values are bit-identical to the es form
            pns = [[None] * RT, [None] * RT]
            for hh in range(2):
                for ri in range(RT):
                    w = (ri + 1) * 128
                    e, m = es_[hh][ri]
                    pn = pupool.tile([128, S], F16, name="pn", tag="pn")
                    nc.vector.scalar_tensor_tensor(
                        pn[:, 0:w], e[:, 0:w],
                        rz[:, ri * 2 + hh:ri * 2 + hh + 1],
                        m[:, 0:w], op0=AO.mult, op1=AO.mult)
                    pns[hh][ri] = pn
            return pns

        def stage_b(hp, pns, vh, use_sc=False):
            """Transposes + attnT for head pair hp (PE-heavy filler)."""
            ptrows = [None] * RT
            for ci in range(RT):
                wv_ = (RT - ci) * 128
                ptb = ps_pt.tile([128, 2 * S], F16, name="ptb", tag="ptb")
                for hh in range(2):
                    for ri in range(ci, RT):
                        o = hh * wv_ + (ri - ci) * 128
                        nc.tensor.transpose(
                            ptb[:, o:o + 128],
                            pns[hh][ri][:, ci * 128:(ci + 1) * 128],
                            identp_sb)
                ptrow = ptpool.tile([128, 2 * S], F16, name="ptrow",
                                    tag="ptrow")
                if ci == 0:
                    nc.scalar.copy(ptrow[:, 0:2 * wv_], ptb[:, 0:2 * wv_])
                else:
                    nc.vector.tensor_copy(ptrow[:, 0:2 * wv_],
                                          ptb[:, 0:2 * wv_])
                ptrows[ci] = ptrow
            # attnT: the two heads' M=64 matmuls hit different column groups
            if use_sc:
                # last unit of the kernel: score ring is idle, use it so
                # this attnT doesn't wait on the aty bank's previous evac
                at_ps = ps_sc.tile([128, S], F32, name="atps", tag="sps")
            else:
                at_ps = ps_aty.tile([128, S], F32, name="atps", tag="aty")
            for ci in range(RT):
                wv_ = (RT - ci) * 128
                for hh in range(2):
                    h = 2 * hp + hh
                    po = hh * 64
                    nc.tensor.matmul(
                        at_ps[po:po + 64, ci * 128:S],
                        vh[ci][:, h * DK:(h + 1) * DK],
                        ptrows[ci][:, hh * wv_:(hh + 1) * wv_],
                        start=(ci == 0), stop=(ci == RT - 1),
                        skip_group_check=True)
            at = atpool.tile([128, S], F16, name=f"at{hp}", tag=f"at{hp}")
            nc.scalar.copy(at, at_ps)
            return at

        def emit_y(b, attnT_sb):
            for rt in range(RT):
                if b == BC - 1:
                    # last batch: the score ring is idle (no more QK) and
                    # its 3 banks let the final y projections pipeline
                    # instead of serializing through the single aty bank
                    yps = ps_sc.tile([128, D], F32, name="yps", tag="sps")
                else:
                    yps = ps_aty.tile([128, D], F32, name="yps", tag="aty")
                nbias = "bo" in bias_sb
                for hp in range(FT):
                    nc.tensor.matmul(
                        yps, attnT_sb[hp][:, rt * 128:(rt + 1) * 128],
                        wo_sb[hp],
                        start=(hp == 0), stop=(hp == FT - 1 and not nbias))
                if nbias:
                    nc.tensor.matmul(
                        yps, ones_sb[0:1, 0:128], bias_sb["bo"],
                        start=False, stop=True)
                y = ypool.tile([128, D], F32, name="y", tag="y")
                nc.scalar.copy(y, yps)
                nc.sync.dma_start(out[b, rt * 128:(rt + 1) * 128, :], y)

        for b in range(BC):
            qhT, khT, vh, do_v = emit_proj(b)
            # software pipeline: while stage_a(hp)'s elementwise chain
            # drains, PE runs v-projection groups, stage_a(hp+1)'s QKs and
            # stage_b(hp-1)'s transposes/attnT
            pns = [None] * FT
            attnT_sb = [None] * FT
            st0 = stage_a1(0, qhT, khT)
            do_v(0)
            st1 = stage_a1(1, qhT, khT)
            do_v(1)
            pns[0] = stage_a2(0, st0)
            do_v(2)
            do_v(3)
            attnT_sb[0] = stage_b(0, pns[0], vh)
            st2 = stage_a1(2, qhT, khT)
            pns[1] = stage_a2(1, st1)
            attnT_sb[1] = stage_b(1, pns[1], vh)
            st3 = stage_a1(3, qhT, khT)
            pns[2] = stage_a2(2, st2)
            attnT_sb[2] = stage_b(2, pns[2], vh)
            pns[3] = stage_a2(3, st3)
            attnT_sb[3] = stage_b(3, pns[3], vh, use_sc=(b == BC - 1))
            emit_y(b, attnT_sb)

    nc.compile()
    return nc


def kernel(**inputs):
    q = np.asarray(inputs["q"], np.float32)
    k = np.asarray(inputs["k"], np.float32)
    v = np.asarray(inputs["v"], np.float32)
    w_q = np.asarray(inputs["w_q"], np.float32)
    w_k = np.asarray(inputs["w_k"], np.float32)
    w_v = np.asarray(inputs["w_v"], np.float32)
    w_o = np.asarray(inputs["w_o"], np.float32)
    b_q = np.asarray(inputs["b_q"], np.float32)
    b_k = np.asarray(inputs["b_k"], np.float32)
    b_v = np.asarray(inputs["b_v"], np.float32)
    b_o = np.asarray(inputs["b_o"], np.float32)
    k_index = int(np.asarray(inputs["k_index"]))
    assert 1 <= k_index <= 8, f"kernel supports k_index<=8, got {k_index}"

    mode = CFG["score_mode"]

    # fold the 1/sqrt(DK) score scaling into the q projection (exact: 2^-3)
    scale = np.float32(1.0 / math.sqrt(DK))
    w_qs = (w_q * scale).astype(np.float32)
    b_qs = (b_q * scale).astype(np.float32)

    has_bias = {
        "bq": bool(np.any(b_qs)),
        "bk": bool(np.any(b_k)),
        "bv": bool(np.any(b_v)),
        "bo": bool(np.any(b_o)),
    }

    nc = _build_program(k_index, has_bias, mode)
    global _last_nc
    _last_nc = nc

    def split16(x):
        hi = x.astype(np.float16)
        lo = (x - hi.astype(np.float32)).astype(np.float16)
        return hi, lo

    shared = {
        "wv": np.ascontiguousarray(w_v.astype(np.float16)),
        "wo": np.ascontiguousarray(w_o.astype(np.float16)),
    }
    if mode == "f16x2":
        for nm, arr in (("wq", w_qs), ("wk", w_k)):
            hi, lo = split16(arr)
            shared[nm + "h"] = np.ascontiguousarray(hi)
            shared[nm + "l"] = np.ascontiguousarray(lo)
    else:
        shared["wq"] = np.ascontiguousarray(w_qs)
        shared["wk"] = np.ascontiguousarray(w_k)
    for nm, arr in (("bq", b_qs), ("bk", b_k), ("bv", b_v), ("bo", b_o)):
        if has_bias[nm]:
            shared[nm] = np.ascontiguousarray(
                arr.reshape(1, D).astype(np.float32))

    in_maps = []
    for c in range(NCORES):
        sl = slice(c * BC, (c + 1) * BC)
        m = dict(shared)
        m["vT"] = np.ascontiguousarray(
            v[sl].transpose(0, 2, 1).astype(np.float16))
        qc = np.ascontiguousarray(q[sl].transpose(0, 2, 1))
        kc = np.ascontiguousarray(k[sl].transpose(0, 2, 1))
        if mode == "f16x2":
            m["qTh"], m["qTl"] = split16(qc)
            m["kTh"], m["kTl"] = split16(kc)
            for nm in ("qTh", "qTl", "kTh", "kTl"):
                m[nm] = np.ascontiguousarray(m[nm])
        else:
            m["qT"], m["kT"] = qc, kc
        in_maps.append(m)

    res = run_bass_kernel_spmd(
        nc, in_maps, core_ids=list(range(NCORES)), trace=CFG["trace"]
    )
    out = np.concatenate([r["out"] for r in res.results], axis=0)
    kernel.last_result = res
    return out
